# revision 26
# baseline (speedup 1.0000x reference)
"""MCR loss kernel for Trainium2 (8 NeuronCores), v3.

Per core: 2 timesteps x 3 feature maps = 6 input planes [32c, 192h, 192w].

  - DMA: 24-row h-slabs, pass A = planes 0-3 on 128 partitions (g,c),
    pass B = planes 4-5 on 64 partitions; 18.4 KB contiguous runs per
    partition, interleaved A/B so the DVE is continuously fed.
  - stage 1 (w-direction 8:1 pool) on DVE: tensor_reduce over the
    contiguous innermost 8, writing an x-major transposed intermediate
    [p, (x24, h24)] so stage 2 also reduces a contiguous axis.
  - stage 2 (h-direction 8:1) on DVE: reduce over r8 (contiguous),
    writing bf16 directly into a reflect-padded x-major conv input
    [p, 26x, 26y]; 4 small edge copies per pad finish the pad.
  - conv: 9 shifted bf16 matmuls (1 cyc/col) with block-diag [96,96]
    stationaries for t0; 27 row/col-tiled matmuls for t1 (its planes
    straddle the two pad buffers). LeakyReLU(0.2) = Act copy + DVE
    max(0.2z, z), output V in bf16 (x-major pixel order; the Gram is
    invariant to pixel order).
  - Gram G_t = V_t V_t^T via bf16 PE transpose + matmul chunks.
  - Host: logdet(I_576 + a V^T V) = logdet(I_96 + a V V^T); float64
    Cholesky on [16,96,96] Grams finishes the scalar loss.
"""

import numpy as np

_STATE = {}

# -------- fixed problem geometry (hardcoded per harness contract) --------
B, CCH, H, W = 16, 32, 192, 192
NCORES = 8
TPC = B // NCORES          # timesteps per core = 2
OUT = 24                   # pooled spatial size
PIX = OUT * OUT            # 576
M = 96                     # feature rows (3 maps x 32 channels)
ALPHA_E = 6.0              # 576 / (96 * eps)
ALPHA_C = 18.0             # 576 / (32 * eps)
PAD = 26                   # padded conv input edge
PPIX = PAD * PAD           # 676
NQ = 8                     # 24-row h-slabs per pass
HR = H // NQ               # 24 rows per slab


def _build_nc():
    import concourse.bass as bass
    import concourse.tile as tile
    from concourse import bacc, mybir

    DT = mybir.dt.float32
    BF = mybir.dt.bfloat16

    nc = bacc.Bacc(
        "TRN2", target_bir_lowering=False, debug=False, num_devices=NCORES
    )

    # x[g] for g = t*3+m : feature-map plane stacks, host-reordered
    x = nc.declare_dram_parameter("x", [TPC * 3, CCH, H, W], DT, isOutput=False)
    # block-diag conv weights: wt[(m,ic), (dy*3+dx)*96 + (m,oc)], bf16
    wt = nc.declare_dram_parameter("wt", [96, 9 * 96], BF, isOutput=False)
    # t1 weights at partitions matching their fmap: m1@0, m2@32, m0@96
    wtt1 = nc.declare_dram_parameter("wtt1", [128, 1440], BF, isOutput=False)
    ident = nc.declare_dram_parameter("ident", [96, 96], BF, isOutput=False)
    g_out = nc.declare_dram_parameter("g_out", [TPC, M, M], DT, isOutput=True)

    with tile.TileContext(nc) as tc:
        with (
            tc.tile_pool(name="persist", bufs=1) as persist,
            tc.tile_pool(name="slabsA", bufs=3) as slabsA,
            tc.tile_pool(name="slabsB", bufs=3) as slabsB,
            tc.tile_pool(name="wsums", bufs=2) as wsumsA,
            tc.tile_pool(name="wsumsB", bufs=2) as wsumsB,
            tc.tile_pool(name="vt", bufs=2) as vtpool,
            tc.tile_pool(name="convps", bufs=2, space="PSUM") as convps,
            tc.tile_pool(name="vtps", bufs=2, space="PSUM") as vtps,
            tc.tile_pool(name="gramps", bufs=1, space="PSUM") as gramps,
        ):
            wt_sb = persist.tile([96, 9 * 96], BF, tag="wt")
            nc.gpsimd.dma_start(out=wt_sb[:], in_=wt.ap())
            wtt1_sb = persist.tile([128, 1440], BF, tag="wtt1")
            nc.gpsimd.dma_start(out=wtt1_sb[:], in_=wtt1.ap())
            id_sb = persist.tile([96, 96], BF, tag="ident")
            nc.gpsimd.dma_start(out=id_sb[:], in_=ident.ap())

            # x-major reflect-padded pooled conv inputs (bf16)
            padA = persist.tile([128, PPIX], BF, tag="padA")  # planes 0-3
            # planes 4-5 at 128 partitions: p = hh*64 + (g-4)*32 + c,
            # hh = y-half; local y cols: interior 1..12, halo/edge 0 and 13
            padB = persist.tile([128, PAD * 14], BF, tag="padB")
            v_sb = persist.tile([96, TPC * PIX], BF, tag="v")
            g_sb = persist.tile([96, TPC * 96], DT, tag="g")

            pA3 = padA[:].rearrange("p (x y) -> p x y", x=PAD)
            pB3 = padB[:].rearrange("p (x y) -> p x y", x=PAD)
            halo = persist.tile([128, 8 * W], DT, tag="halo")

            # ---- pooling helpers ----
            def slab_dma_A(h0, nr):
                slab = slabsA.tile([128, nr * W], DT, tag="slabA")
                nc.sync.dma_start(
                    out=slab[:, : nr * W],
                    in_=x.ap()[0:4, :, h0 : h0 + nr, :].rearrange(
                        "g c h w -> (g c) (h w)"
                    ),
                )
                return slab

            def slab_reduce_A(h0, nr, slab):
                y0 = h0 // 8
                wsum = wsumsA.tile([128, HR * OUT], DT, tag="wsA")
                nc.vector.tensor_reduce(
                    out=wsum[:, : nr * OUT].rearrange("p (x h) -> p h x", h=nr),
                    in_=slab[:, : nr * W].rearrange(
                        "p (h x b) -> p h x b", x=OUT, b=8
                    ),
                    axis=mybir.AxisListType.X,
                    op=mybir.AluOpType.add,
                )
                with nc.allow_low_precision(
                    reason="pooled conv input in bf16 (tol 2e-2)"
                ):
                    nc.vector.tensor_reduce(
                        out=pA3[:, 1:25, y0 + 1 : y0 + 1 + nr // 8],
                        in_=wsum[:, : nr * OUT].rearrange(
                            "p (x hg r) -> p x hg r", hg=nr // 8, r=8
                        ),
                        axis=mybir.AxisListType.X,
                        op=mybir.AluOpType.add,
                    )

            def slab_dma_B(j, nr):
                # partitions (hh, g, c); hh half h = hh*96 + 8*j .. +nr
                slab = slabsB.tile([128, nr * W], DT, tag="slabB")
                for hh in range(2):
                    for gi in range(2):
                        nc.sync.dma_start(
                            out=slab[
                                hh * 64 + gi * 32 : hh * 64 + gi * 32 + 32,
                                : nr * W,
                            ].rearrange("c (h w) -> c h w", h=nr),
                            in_=x.ap()[
                                4 + gi, :, hh * 96 + 8 * j : hh * 96 + 8 * j + nr, :
                            ],
                        )
                return slab

            def slab_reduce_B(j, nr, slab):
                # local y cols: y+1 (hh0) / y-11 (hh1); both halves of a
                # slab land on the same local cols j+1 .. j+nr//8
                wsum = wsumsB.tile([128, 16 * OUT], DT, tag="wsB")
                nc.vector.tensor_reduce(
                    out=wsum[:, : nr * OUT].rearrange("p (x h) -> p h x", h=nr),
                    in_=slab[:, : nr * W].rearrange(
                        "p (h x b) -> p h x b", x=OUT, b=8
                    ),
                    axis=mybir.AxisListType.X,
                    op=mybir.AluOpType.add,
                )
                with nc.allow_low_precision(
                    reason="pooled conv input in bf16 (tol 2e-2)"
                ):
                    nc.vector.tensor_reduce(
                        out=pB3[:, 1:25, j + 1 : j + 1 + nr // 8],
                        in_=wsum[:, : nr * OUT].rearrange(
                            "p (x hg r) -> p x hg r", hg=nr // 8, r=8
                        ),
                        axis=mybir.AxisListType.X,
                        op=mybir.AluOpType.add,
                    )

            def halo_dma():
                # hh0 partitions get h 96..103 (pooled y12 -> local col 13),
                # hh1 partitions get h 88..95 (pooled y11 -> local col 0)
                for tgt, h0 in ((0, 96), (1, 88)):
                    for gi in range(2):
                        nc.sync.dma_start(
                            out=halo[
                                tgt * 64 + gi * 32 : tgt * 64 + gi * 32 + 32, :
                            ].rearrange("c (h w) -> c h w", h=8),
                            in_=x.ap()[4 + gi, :, h0 : h0 + 8, :],
                        )

            def halo_reduce():
                hwsum = wsumsB.tile([128, 16 * OUT], DT, tag="wsB")
                nc.vector.tensor_reduce(
                    out=hwsum[:, :192].rearrange("p (x h) -> p h x", h=8),
                    in_=halo[:].rearrange("p (h x b) -> p h x b", x=OUT, b=8),
                    axis=mybir.AxisListType.X,
                    op=mybir.AluOpType.add,
                )
                with nc.allow_low_precision(
                    reason="pooled conv input in bf16 (tol 2e-2)"
                ):
                    nc.vector.tensor_reduce(
                        out=pB3[0:64, 1:25, 13:14],
                        in_=hwsum[0:64, :192].rearrange(
                            "p (x hg r) -> p x hg r", hg=1, r=8
                        ),
                        axis=mybir.AxisListType.X,
                        op=mybir.AluOpType.add,
                    )
                    nc.vector.tensor_reduce(
                        out=pB3[64:128, 1:25, 0:1],
                        in_=hwsum[64:128, :192].rearrange(
                            "p (x hg r) -> p x hg r", hg=1, r=8
                        ),
                        axis=mybir.AxisListType.X,
                        op=mybir.AluOpType.add,
                    )

            def fixA0():
                nc.vector.tensor_copy(pA3[:, 0:1, 1:14], pA3[:, 2:3, 1:14])
                nc.vector.tensor_copy(pA3[:, 25:26, 1:14], pA3[:, 23:24, 1:14])
                nc.vector.tensor_copy(pA3[:, :, 0:1], pA3[:, :, 2:3])

            def fixA1():
                nc.vector.tensor_copy(pA3[:, 0:1, 14:25], pA3[:, 2:3, 14:25])
                nc.vector.tensor_copy(pA3[:, 25:26, 14:25], pA3[:, 23:24, 14:25])
                nc.vector.tensor_copy(pA3[:, :, 25:26], pA3[:, :, 23:24])

            def fixB_edges0():
                # hh0 local col 0 = reflect of its local col 2 (pooled y1)
                nc.vector.tensor_copy(pB3[0:64, 1:25, 0:1], pB3[0:64, 1:25, 2:3])

            def fixB_final():
                # hh1 local col 13 = reflect of its local col 11 (pooled
                # y22), then x-edge rows over all 14 local cols
                nc.vector.tensor_copy(
                    pB3[64:128, 1:25, 13:14], pB3[64:128, 1:25, 11:12]
                )
                nc.vector.tensor_copy(pB3[:, 0:1, :], pB3[:, 2:3, :])
                nc.vector.tensor_copy(pB3[:, 25:26, :], pB3[:, 23:24, :])

            dydx = [(a, b) for a in range(3) for b in range(3)]
            pcs = {}

            def conv_mms_t0(y0, ny):
                pcb = convps.tile([96, 512], DT, tag="convps")
                pc = pcb[:, 0 : ny * 24]
                for i, (dy, dx) in enumerate(dydx):
                    blk = dy * 3 + dx
                    nc.tensor.matmul(
                        pc[:],
                        wt_sb[:, blk * 96 : (blk + 1) * 96],
                        pA3[:96, dx : dx + 24, y0 + dy : y0 + dy + ny
                            ].transpose([0, 2, 1]),
                        start=(i == 0), stop=(i == 8),
                    )
                zc = vtpool.tile([96, 288], DT, tag="zcopy")
                nc.scalar.copy(zc[:, : ny * 24], pc[:])
                pcs[(0, y0)] = (pc, zc)

            def conv_mms_t1_m0(yh):
                # t1 V rows (m1, m2, m0); m0 strip from pA3[96:128]
                pcb = convps.tile([96, 512], DT, tag="convps")
                pc = pcb[:, 0:288]
                pcs[(1, yh * 12)] = pc
                for i, (dy, dx) in enumerate(dydx):
                    blk = dy * 3 + dx
                    nc.tensor.matmul(
                        pc[64:96, :],
                        wtt1_sb[96:128, 1152 + blk * 32 : 1152 + blk * 32 + 32],
                        pA3[96:128, dx : dx + 24,
                            yh * 12 + dy : yh * 12 + dy + 12
                            ].transpose([0, 2, 1]),
                        start=(i == 0), stop=(i == 8),
                        tile_position=(96, 64),
                    )

            def conv_mms_t1_m12(yh):
                pc = pcs[(1, yh * 12)]
                for i, (dy, dx) in enumerate(dydx):
                    blk = dy * 3 + dx
                    nc.tensor.matmul(
                        pc[0:64, :],
                        wtt1_sb[
                            64 * yh : 64 * yh + 64,
                            yh * 576 + blk * 64 : yh * 576 + blk * 64 + 64,
                        ],
                        pB3[64 * yh : 64 * yh + 64, dx : dx + 24,
                            dy : dy + 12].transpose([0, 2, 1]),
                        start=(i == 0), stop=(i == 8),
                    )
                zc = vtpool.tile([96, 288], DT, tag="zcopy")
                nc.scalar.copy(zc[:], pc[:])
                pcs[(1, yh * 12)] = (pc, zc)

            def relu(t, y0, ny):
                pc, zc = pcs[(t, y0)]
                nc.vector.scalar_tensor_tensor(
                    out=v_sb[:, t * PIX + y0 * 24 : t * PIX + (y0 + ny) * 24],
                    in0=zc[:, : ny * 24],
                    scalar=0.2,
                    in1=pc[:],
                    op0=mybir.AluOpType.mult,
                    op1=mybir.AluOpType.max,
                )

            vt_alls = {}

            def gram_chunks(t, chunks):
                if t not in vt_alls:
                    vt_all = vtpool.tile([128, 5 * 96], BF, tag=f"vtall{t}")
                    vt_alls[t] = vt_all
                vt_all = vt_alls[t]
                for c in chunks:
                    sz = 128 if c < 4 else 64
                    vslice = v_sb[:, t * PIX + c * 128 : t * PIX + c * 128 + sz]
                    ptb = vtps.tile([128, 1024], BF, tag="vtps")
                    pt = ptb[:, 0:96]
                    nc.tensor.transpose(pt[:sz, :], vslice, id_sb[:])
                    nc.scalar.copy(vt_all[:sz, c * 96 : (c + 1) * 96], pt[:sz, :])

            def gram_finish(t, dmae):
                vt_all = vt_alls[t]
                gpb = gramps.tile([96, 512], DT, tag="gram")
                gp = gpb[:, 0:96]
                for c in range(5):
                    sz = 128 if c < 4 else 64
                    nc.tensor.matmul(
                        gp[:],
                        vt_all[:sz, c * 96 : (c + 1) * 96],
                        vt_all[:sz, c * 96 : (c + 1) * 96],
                        start=(c == 0), stop=(c == 4),
                    )
                nc.scalar.copy(g_sb[:, t * 96 : (t + 1) * 96], gp[:])
                dmae.dma_start(
                    out=g_out[t], in_=g_sb[:, t * 96 : (t + 1) * 96]
                )

            # ---- schedule ----
            # A: 8x 24-row slabs (t0 + t1's m0 plane); B: 6x 16-row slabs
            # at 128 partitions (hh,g,c) + an 8-row halo. t0 streams; t1's
            # m0 strip streams; t1's m1/m2 + gram run after the B tail.
            Aseq = [("A", h) for h in range(0, 192, 24)]
            seq = ["B0", Aseq[0], Aseq[1], "halo", "B1", Aseq[2], "B2",
                   Aseq[3], "B3", Aseq[4], "W0", Aseq[5], "B4", "W0r",
                   Aseq[6], Aseq[7], "W2a", "B5a", "W2b", "B5b", "TAIL"]
            for step in seq:
                if step == "B0":
                    slab_reduce_B(0, 16, slab_dma_B(0, 16))
                    fixB_edges0()
                elif step == "halo":
                    halo_dma()
                    halo_reduce()
                elif step in ("B1", "B2", "B3", "B4"):
                    j = 2 * int(step[1])
                    slab_reduce_B(j, 16, slab_dma_B(j, 16))
                elif step == "B5a":
                    slab_reduce_B(10, 8, slab_dma_B(10, 8))
                elif step == "B5b":
                    slab_reduce_B(11, 8, slab_dma_B(11, 8))
                elif step == "W0":
                    fixA0()
                    conv_mms_t0(0, 12)
                elif step == "W0r":
                    relu(0, 0, 12)
                    gram_chunks(0, [0, 1])
                elif step == "W2a":
                    fixA1()
                    conv_mms_t0(12, 12)
                    conv_mms_t1_m0(0)
                    conv_mms_t1_m0(1)
                elif step == "W2b":
                    relu(0, 12, 12)
                    gram_chunks(0, [2, 3, 4])
                    gram_finish(0, nc.gpsimd)
                elif step == "TAIL":
                    fixB_final()
                    conv_mms_t1_m12(0)
                    conv_mms_t1_m12(1)
                    relu(1, 0, 12)
                    relu(1, 12, 12)
                    gram_chunks(1, [0, 1, 2, 3, 4])
                    gram_finish(1, nc.sync)
                else:
                    _, h = step
                    slab_reduce_A(h, 24, slab_dma_A(h, 24))

    nc.finalize()
    return nc


def _get_nc():
    if "nc" not in _STATE:
        _STATE["nc"] = _build_nc()
    return _STATE["nc"]


def _prep_weights(W1, W2, W3):
    import ml_dtypes

    # wt[(m,ic), (dy*3+dx)*96 + 32m+oc] = W_m[oc, ic, dy, dx] / 64
    wt = np.zeros((96, 9 * 96), dtype=np.float64)
    for m, Wm in enumerate((W1, W2, W3)):
        Wm = np.asarray(Wm, np.float64) / 64.0  # [oc, ic, dy, dx]
        for dy in range(3):
            for dx in range(3):
                blk = dy * 3 + dx
                wt[
                    32 * m : 32 * m + 32,
                    blk * 96 + 32 * m : blk * 96 + 32 * m + 32,
                ] = Wm[:, :, dy, dx].T
    # wtt1 for t1 (V rows ordered m1, m2, m0), partition-aligned with the
    # fmaps: cols 0..575 m1+m2 blocks at rows 0..63 (yh0), cols 576..1151
    # the same blocks at rows 64..127 (yh1), cols 1152.. m0 (W1) at rows
    # 96..127
    wtt1 = np.zeros((128, 1440), dtype=np.float64)
    w1 = np.asarray(W1, np.float64) / 64.0
    w2 = np.asarray(W2, np.float64) / 64.0
    w3 = np.asarray(W3, np.float64) / 64.0
    for dy in range(3):
        for dx in range(3):
            blk = dy * 3 + dx
            for yh in range(2):
                o, r = yh * 576, yh * 64
                wtt1[r : r + 32, o + blk * 64 : o + blk * 64 + 32] = (
                    w2[:, :, dy, dx].T
                )
                wtt1[r + 32 : r + 64, o + blk * 64 + 32 : o + blk * 64 + 64] = (
                    w3[:, :, dy, dx].T
                )
            wtt1[96:128, 1152 + blk * 32 : 1152 + blk * 32 + 32] = (
                w1[:, :, dy, dx].T
            )
    bf = ml_dtypes.bfloat16
    return wt.astype(np.float32).astype(bf), wtt1.astype(np.float32).astype(bf)


def _host_loss(G):
    G = np.asarray(G, np.float64)  # [16, 96, 96]
    T = G.shape[0]
    I96 = np.eye(M)
    Me = I96[None] + ALPHA_E * G
    ld_e = 2.0 * np.log(
        np.diagonal(np.linalg.cholesky(Me), axis1=-2, axis2=-1)
    ).sum()
    blocks = np.stack(
        [G[:, 32 * c : 32 * (c + 1), 32 * c : 32 * (c + 1)] for c in range(3)]
    )  # [3, T, 32, 32]
    Mc = np.eye(32)[None, None] + ALPHA_C * blocks
    ld_c = 2.0 * np.log(
        np.diagonal(np.linalg.cholesky(Mc), axis1=-2, axis2=-1)
    ).sum()
    loss_expd = ld_e / (2.0 * T)
    loss_comp = (32.0 / M) * ld_c / (2.0 * T)
    return np.float32(loss_expd - loss_comp)


def run_device(inputs, **kw):
    """Run the bass kernel; returns (G [16,96,96], BassKernelResults)."""
    import ml_dtypes
    from concourse.bass_utils import run_bass_kernel_spmd

    nc = _get_nc()
    wt, wtt1 = _prep_weights(inputs["W1"], inputs["W2"], inputs["W3"])
    ident = np.eye(96, dtype=np.float32).astype(ml_dtypes.bfloat16)
    ms = np.asarray(inputs["ms_fea"], np.float32)
    pan = np.asarray(inputs["pan_fea"], np.float32)
    alf = np.asarray(inputs["all_fea"], np.float32)
    in_maps = []
    for i in range(NCORES):
        sl = slice(TPC * i, TPC * (i + 1))
        # x[t*3+m] = (ms,pan,alf)[m][t]
        xs = np.stack([ms[sl], pan[sl], alf[sl]], axis=1).reshape(
            TPC * 3, CCH, H, W
        )
        in_maps.append(
            {"x": np.ascontiguousarray(xs), "wt": wt, "wtt1": wtt1,
             "ident": ident}
        )
    res = run_bass_kernel_spmd(nc, in_maps, core_ids=list(range(NCORES)), **kw)
    G = np.concatenate([np.asarray(r["g_out"]) for r in res.results], axis=0)
    # odd timesteps were computed with V rows ordered (m1, m2, m0)
    perm = np.r_[64:96, 0:32, 32:64]
    G[1::2] = G[1::2][:, perm][:, :, perm]
    return G, res


def kernel(**inputs):
    G, _ = run_device(inputs)
    return _host_loss(G)
